# revision 5
# baseline (speedup 1.0000x reference)
"""Trainium2 Bass kernel for sheaf Dirichlet energy (ConsistencyBasedLaplacianBuilder).

loss = sum_e || maps[rev(e)] @ x[tgt(e)] - maps[e] @ x[src(e)] ||_F^2

Strategy (edge parallelism across 8 NeuronCores):
  The edge set is symmetric (rev(e) = e +- H), so
  loss = 2 * sum_{e<H} ||maps[e+H] x[dst] - maps[e] x[src]||^2.
  Each core takes 100k half-edges.

  x is packed into bf16 pair-rows xpair[r] = [x[2r] | x[2r+1]] (256B rows)
  so a single int16-indexed dma_gather (idx = node>>1) fetches each
  endpoint; which 64-element half holds the wanted node is the node's
  parity. Edges are partitioned on the host into 4 parity classes
  (dst&1, src&1) occupying fixed tile ranges, so the parity offsets are
  compile-time constants in the access patterns.

  Per group of 8 tiles (1024 edges): one 2048-row gather (dst+src rows,
  queue round-robin over the 4 SWDGE queues so descriptor generation
  uses all Q7 core pairs), then on DVE per tile
    prod[e, i, jj, f] = mcat[e, i, jj] * xcat[e, jj, f]   (bf16)
  with mcat = [A | -B] host-prepared, followed by group-wide bf16 tree
  adds over jj (128->64->32->16 wide), and Square+accumulate on the
  Scalar engine. Per-core scalars are summed on the host.
"""

import sys
import types

import numpy as np

sys.path.insert(0, "/opt/trn_rl_repo")

N = 50000
D = 4
F = 16
DF = D * F            # 64 floats per node row
E = 1600000
H = E // 2            # 800000 undirected pairs
NCORES = 8
EPC = H // NCORES     # 100000 half-edges per core

NPAIR = N // 2 + 88   # 25088 bf16 pair rows (256B each), zero padded
GROUP = 8             # tiles per gather group
CB_G = 27             # groups per parity class
CB_EDGES = CB_G * GROUP * 128   # 26624 edge slots per class
NG = 4 * CB_G         # 104 groups per core
NT = NG * GROUP       # 832 tiles per core
NQ = 4                # SWDGE queues


def _inject_axon_hooks():
    """Provide antenv.axon_hooks if missing so NTFF tracing can register."""
    if "antenv.axon_hooks" in sys.modules:
        return
    try:
        import antenv.axon_hooks  # noqa: F401
        return
    except Exception:
        pass
    mod = types.ModuleType("antenv.axon_hooks")
    mod._hook = None

    def set_axon_ntff_profile_hook(h):
        mod._hook = h

    def get_axon_ntff_profile_hook():
        return mod._hook

    mod.set_axon_ntff_profile_hook = set_axon_ntff_profile_hook
    mod.get_axon_ntff_profile_hook = get_axon_ntff_profile_hook
    sys.modules["antenv.axon_hooks"] = mod


def _build_program():
    import concourse.bacc as bacc
    import concourse.bass as bass
    import concourse.tile as tile
    from concourse import mybir

    AP = bass.AP
    f32 = mybir.dt.float32
    bf16 = mybir.dt.bfloat16
    i16 = mybir.dt.int16
    Op = mybir.AluOpType
    Act = mybir.ActivationFunctionType
    ds = bass.ds

    nc = bacc.Bacc("TRN2", target_bir_lowering=False, debug=False,
                   num_devices=NCORES, num_swdge_queues=NQ)

    xpair_d = nc.dram_tensor("xpair", [NPAIR, 2 * DF], bf16,
                             kind="ExternalInput")
    mcat_d = nc.dram_tensor("mcat", [128, NT * 32], bf16,
                            kind="ExternalInput")
    gidx_d = nc.dram_tensor("gidx", [128, NG * 128], i16,
                            kind="ExternalInput")
    loss_d = nc.dram_tensor("loss", [1, 1], f32, kind="ExternalOutput")

    NBUF = 12

    with tile.TileContext(nc) as tc, \
         tc.tile_pool(name="persist", bufs=1) as pp, \
         tc.tile_pool(name="work", bufs=2) as wp, \
         tc.tile_pool(name="psum", bufs=1, space="PSUM") as psp:

        mcat_sb = pp.tile([128, NT * 32], bf16, tag="mcat")
        gidx_sb = pp.tile([128, NG * 128], i16, tag="gidx")
        acc = pp.tile([128, NG], f32, tag="acc")

        nc.sync.dma_start(gidx_sb[:], gidx_d[:])
        nc.sync.dma_start(mcat_sb[:], mcat_d[:])

        dbufs = [pp.tile([128, GROUP * 2 * 2 * DF], bf16, tag=f"db{i}",
                         name=f"db{i}") for i in range(NBUF)]

        def gather(g):
            db = dbufs[g % NBUF]
            b = db[:]
            out3 = AP(b.tensor, b.offset,
                      [b.ap[0], [2 * DF, 2 * GROUP], [1, 2 * DF]])
            nc.gpsimd.dma_gather(
                out_ap=out3, in_ap=xpair_d[:],
                idxs_ap=gidx_sb[:, ds(g * 128, 128)],
                num_idxs=2 * GROUP * 128, num_idxs_reg=2 * GROUP * 128,
                elem_size=2 * DF, single_packet=False,
                queue_num=1 + g % (NQ - 1))

        def compute(g):
            q = g // CB_G
            pd, ps = q >> 1, q & 1
            hstride = 2 * DF + DF * (ps - pd)
            db = dbufs[g % NBUF]
            prod = wp.tile([128, GROUP * 512], bf16, tag="prod")
            t1 = wp.tile([128, GROUP * 256], bf16, tag="t1")
            t2 = wp.tile([128, GROUP * 128], bf16, tag="t2")
            dd = wp.tile([128, GROUP * 64], bf16, tag="dd")
            sq = wp.tile([128, GROUP * 64], bf16, tag="sq")

            b = db[:]
            m0 = mcat_sb[:]
            p0 = prod[:]
            for t in range(GROUP):
                in0 = AP(b.tensor, b.offset + 4 * DF * t + DF * pd,
                         [b.ap[0], [0, D], [hstride, 2], [1, DF]])
                in1 = AP(m0.tensor, m0.offset + 32 * (g * GROUP + t),
                         [m0.ap[0], [8, D], [1, 8], [0, F]])
                po = AP(p0.tensor, p0.offset + 512 * t,
                        [p0.ap[0], [128, D], [DF, 2], [1, DF]])
                nc.vector.tensor_tensor(po, in0, in1, Op.mult)

            # tree-reduce over jj: per (tile,i) 128-block: h halves, then jl
            a0 = AP(p0.tensor, p0.offset, [p0.ap[0], [128, 32], [1, 64]])
            a1 = AP(p0.tensor, p0.offset + 64, [p0.ap[0], [128, 32], [1, 64]])
            t1v = t1[:]
            o1 = AP(t1v.tensor, t1v.offset, [t1v.ap[0], [64, 32], [1, 64]])
            nc.vector.tensor_tensor(o1, a0, a1, Op.add)

            b0 = AP(t1v.tensor, t1v.offset, [t1v.ap[0], [64, 32], [1, 32]])
            b1 = AP(t1v.tensor, t1v.offset + 32,
                    [t1v.ap[0], [64, 32], [1, 32]])
            t2v = t2[:]
            o2 = AP(t2v.tensor, t2v.offset, [t2v.ap[0], [32, 32], [1, 32]])
            nc.vector.tensor_tensor(o2, b0, b1, Op.add)

            c0 = AP(t2v.tensor, t2v.offset, [t2v.ap[0], [32, 32], [1, 16]])
            c1 = AP(t2v.tensor, t2v.offset + 16,
                    [t2v.ap[0], [32, 32], [1, 16]])
            ddv = dd[:]
            o3 = AP(ddv.tensor, ddv.offset, [ddv.ap[0], [16, 32], [1, 16]])
            nc.vector.tensor_tensor(o3, c0, c1, Op.add)

            nc.scalar.activation(sq[:], dd[:], Act.Square,
                                 accum_out=acc[:, g:g + 1])

        for g in range(NG):
            gather(g)
            if g >= 1:
                compute(g - 1)
        compute(NG - 1)

        colsum = pp.tile([128, 1], f32, tag="colsum")
        ones = pp.tile([128, 1], f32, tag="ones")
        nc.vector.reduce_sum(out=colsum[:], in_=acc[:],
                             axis=mybir.AxisListType.X)
        nc.vector.memset(ones[:], 1.0)
        pt = psp.tile([1, 1], f32, tag="pt")
        nc.tensor.matmul(pt[:], lhsT=colsum[:], rhs=ones[:],
                         start=True, stop=True)
        lsb = pp.tile([1, 1], f32, tag="lsb")
        # *2: each undirected pair contributes both directed edges equally
        nc.vector.tensor_scalar(lsb[:], pt[:], 2.0, None, Op.mult)
        nc.sync.dma_start(loss_d[:], lsb[:])

    nc.compile()
    return nc


_CACHED = {}


def _get_program():
    if "nc" not in _CACHED:
        _inject_axon_hooks()
        _CACHED["nc"] = _build_program()
    return _CACHED["nc"]


def _bf16(a):
    import ml_dtypes
    return a.astype(ml_dtypes.bfloat16)


def _prep_core_inputs(maps3d, src, dst, core):
    """Per-core layout transforms. Returns dict or None if class overflow."""
    e0 = core * EPC
    e1 = e0 + EPC
    d = dst[e0:e1]
    s = src[e0:e1]
    A = maps3d[H + e0:H + e1]
    B = maps3d[e0:e1]

    cls = (d & 1) * 2 + (s & 1)
    eidx = np.full(NT * 128, -1, np.int64)
    for q in range(4):
        iq = np.flatnonzero(cls == q)
        if len(iq) > CB_EDGES:
            return None
        eidx[q * CB_EDGES:q * CB_EDGES + len(iq)] = iq
    valid = eidx >= 0
    ev = eidx[valid]

    m8 = np.zeros((NT * 128, D, 8), np.float32)
    m8[valid, :, :4] = A[ev]
    m8[valid, :, 4:] = -B[ev]
    mcat = _bf16(m8.reshape(NT, 128, 32).transpose(1, 0, 2)
                 .reshape(128, NT * 32))

    dstP = np.zeros(NT * 128, np.int64)
    dstP[valid] = d[ev]
    srcP = np.zeros(NT * 128, np.int64)
    srcP[valid] = s[ev]
    lin = np.empty((NT, 2, 128), np.int16)
    lin[:, 0, :] = (dstP >> 1).reshape(NT, 128)
    lin[:, 1, :] = (srcP >> 1).reshape(NT, 128)
    gidx = np.tile(lin.reshape(-1, 16).T, (8, 1))

    return {
        "mcat": np.ascontiguousarray(mcat),
        "gidx": np.ascontiguousarray(gidx),
    }


def _make_in_maps(x, restriction_maps, edge_index):
    """Build per-core input maps (shared xpair included). None on overflow."""
    x_flat = x.reshape(N, DF).astype(np.float32)
    xp = np.zeros((NPAIR, 2 * DF), np.float32)
    xp[:N // 2, :DF] = x_flat[0::2]
    xp[:N // 2, DF:] = x_flat[1::2]
    xpair = _bf16(xp)
    maps3d = restriction_maps.astype(np.float32)
    src = edge_index[0].astype(np.int64)
    dst = edge_index[1].astype(np.int64)
    in_maps = []
    for c in range(NCORES):
        m = _prep_core_inputs(maps3d, src, dst, c)
        if m is None:
            return None
        m["xpair"] = xpair
        in_maps.append(m)
    return in_maps


def _symmetric_structure(rev_idx):
    r = np.asarray(rev_idx)
    if r.shape != (E,):
        return False
    h = np.arange(H, dtype=r.dtype)
    return bool(np.array_equal(r[:H], h + H) and np.array_equal(r[H:], h))


def _fallback_numpy(x, restriction_maps, edge_index, rev_idx):
    x = np.asarray(x, np.float32)
    maps = np.asarray(restriction_maps, np.float32)
    ei = np.asarray(edge_index)
    rv = np.asarray(rev_idx)
    total = np.float64(0.0)
    chunk = 131072
    ne = ei.shape[1]
    for st in range(0, ne, chunk):
        e = min(st + chunk, ne)
        srcc = ei[0, st:e]
        tgt = ei[1, st:e]
        fvu = maps[rv[st:e]]
        fuv = maps[st:e]
        t1 = np.einsum("eij,ejf->eif", fvu, x[tgt])
        t2 = np.einsum("eij,ejf->eif", fuv, x[srcc])
        dd = t1 - t2
        total += np.sum((dd * dd).astype(np.float64))
    return np.float32(total)


def kernel(x, restriction_maps, edge_index, rev_idx):
    x = np.asarray(x)
    restriction_maps = np.asarray(restriction_maps)
    edge_index = np.asarray(edge_index)
    rev_idx = np.asarray(rev_idx)

    if (x.shape != (N, D, F) or restriction_maps.shape != (E, D, D)
            or edge_index.shape != (2, E) or not _symmetric_structure(rev_idx)):
        return _fallback_numpy(x, restriction_maps, edge_index, rev_idx)

    in_maps = _make_in_maps(x, restriction_maps, edge_index)
    if in_maps is None:
        return _fallback_numpy(x, restriction_maps, edge_index, rev_idx)

    from concourse.bass_utils import run_bass_kernel_spmd

    nc = _get_program()
    res = run_bass_kernel_spmd(nc, in_maps, core_ids=list(range(NCORES)))
    total = np.float32(0.0)
    for c in range(NCORES):
        total += res.results[c]["loss"][0, 0]
    return np.float32(total)


# revision 9
# speedup vs baseline: 1.0652x; 1.0652x over previous
"""Trainium2 Bass kernel for sheaf Dirichlet energy (ConsistencyBasedLaplacianBuilder).

loss = sum_e || maps[rev(e)] @ x[tgt(e)] - maps[e] @ x[src(e)] ||_F^2

Strategy (edge parallelism across 8 NeuronCores):
  The edge set is symmetric (rev(e) = e +- H), so
  loss = 2 * sum_{e<H} ||maps[e+H] x[dst] - maps[e] x[src]||^2.
  Each core takes 100k half-edges.

  x is packed into bf16 pair-rows xpair[r] = [x[2r] | x[2r+1]] (256B rows)
  so a single int16-indexed dma_gather (idx = node>>1) fetches each
  endpoint; which 64-element half holds the wanted node is the node's
  parity. Edges are partitioned on the host into 4 parity classes
  (dst&1, src&1) occupying fixed tile ranges, so the parity offsets are
  compile-time constants in the access patterns.

  Per group of 8 tiles (1024 edges): one 2048-row gather (dst+src rows,
  queue round-robin over the 4 SWDGE queues so descriptor generation
  uses all Q7 core pairs), then on DVE per tile
    prod[e, i, jj, f] = mcat[e, i, jj] * xcat[e, jj, f]   (bf16)
  with mcat = [A | -B] host-prepared, followed by group-wide bf16 tree
  adds over jj (128->64->32->16 wide), and Square+accumulate on the
  Scalar engine. Per-core scalars are summed on the host.
"""

import sys
import types

import numpy as np

sys.path.insert(0, "/opt/trn_rl_repo")

N = 50000
D = 4
F = 16
DF = D * F            # 64 floats per node row
E = 1600000
H = E // 2            # 800000 undirected pairs
NCORES = 8
EPC = H // NCORES     # 100000 half-edges per core

NPAIR = N // 2 + 88   # 25088 bf16 pair rows (256B each), zero padded
GROUP = 8             # tiles per gather group
CB_G = 27             # groups per parity class
CB_EDGES = CB_G * GROUP * 128   # 26624 edge slots per class
NG = 4 * CB_G         # 104 groups per core
NT = NG * GROUP       # 832 tiles per core
NQ = 4                # SWDGE queues


def _inject_axon_hooks():
    """Provide antenv.axon_hooks if missing so NTFF tracing can register."""
    if "antenv.axon_hooks" in sys.modules:
        return
    try:
        import antenv.axon_hooks  # noqa: F401
        return
    except Exception:
        pass
    mod = types.ModuleType("antenv.axon_hooks")
    mod._hook = None

    def set_axon_ntff_profile_hook(h):
        mod._hook = h

    def get_axon_ntff_profile_hook():
        return mod._hook

    mod.set_axon_ntff_profile_hook = set_axon_ntff_profile_hook
    mod.get_axon_ntff_profile_hook = get_axon_ntff_profile_hook
    sys.modules["antenv.axon_hooks"] = mod


def _build_program():
    import concourse.bacc as bacc
    import concourse.bass as bass
    import concourse.tile as tile
    from concourse import mybir

    AP = bass.AP
    f32 = mybir.dt.float32
    bf16 = mybir.dt.bfloat16
    i16 = mybir.dt.int16
    Op = mybir.AluOpType
    Act = mybir.ActivationFunctionType
    ds = bass.ds

    nc = bacc.Bacc("TRN2", target_bir_lowering=False, debug=False,
                   num_devices=NCORES, num_swdge_queues=NQ)

    xpair_d = nc.dram_tensor("xpair", [NPAIR, 2 * DF], bf16,
                             kind="ExternalInput")
    mcat_d = nc.dram_tensor("mcat", [128, NT * 32], bf16,
                            kind="ExternalInput")
    gidx_d = nc.dram_tensor("gidx", [128, NG * 128], i16,
                            kind="ExternalInput")
    loss_d = nc.dram_tensor("loss", [1, 1], f32, kind="ExternalOutput")

    NBUF = 8
    GT = 2 * GROUP        # tiles per gather (2 compute groups)
    NGATH = NT // GT      # 54 gathers per core
    GIDX = GT * 256 // 16  # idx columns per gather

    with tile.TileContext(nc) as tc, \
         tc.tile_pool(name="persist", bufs=1) as pp, \
         tc.tile_pool(name="work", bufs=2) as wp, \
         tc.tile_pool(name="psum", bufs=1, space="PSUM") as psp:

        mcat_sb = pp.tile([128, NT * 32], bf16, tag="mcat")
        gidx_sb = pp.tile([128, NG * 128], i16, tag="gidx")
        acc = pp.tile([128, NG], f32, tag="acc")

        nc.sync.dma_start(gidx_sb[:], gidx_d[:])
        nc.sync.dma_start(mcat_sb[:], mcat_d[:])

        dbufs = [pp.tile([128, GT * 2 * 2 * DF], bf16, tag=f"db{i}",
                         name=f"db{i}") for i in range(NBUF)]

        nreg = nc.gpsimd.to_reg(GT * 256)

        def gather(k):
            db = dbufs[k % NBUF]
            b = db[:]
            out3 = AP(b.tensor, b.offset,
                      [b.ap[0], [2 * DF, 2 * GT], [1, 2 * DF]])
            nc.gpsimd.dma_gather(
                out_ap=out3, in_ap=xpair_d[:],
                idxs_ap=gidx_sb[:, ds(k * GIDX, GIDX)],
                num_idxs=GT * 256, num_idxs_reg=nreg,
                elem_size=2 * DF, single_packet=False, queue_num=k % NQ)

        def compute(g):
            q = g // CB_G
            pd, ps = q >> 1, q & 1
            hstride = 2 * DF + DF * (ps - pd)
            db = dbufs[(g // 2) % NBUF]
            toff = (g % 2) * GROUP
            prod = wp.tile([128, GROUP * 512], bf16, tag="prod")
            t1 = wp.tile([128, GROUP * 256], bf16, tag="t1")
            t2 = wp.tile([128, GROUP * 128], bf16, tag="t2")
            dd = wp.tile([128, GROUP * 64], bf16, tag="dd")
            sq = wp.tile([128, GROUP * 64], bf16, tag="sq")

            b = db[:]
            m0 = mcat_sb[:]
            p0 = prod[:]
            for t in range(GROUP):
                in0 = AP(b.tensor, b.offset + 4 * DF * (toff + t) + DF * pd,
                         [b.ap[0], [0, D], [hstride, 2], [1, DF]])
                in1 = AP(m0.tensor, m0.offset + 32 * (g * GROUP + t),
                         [m0.ap[0], [8, D], [1, 8], [0, F]])
                po = AP(p0.tensor, p0.offset + 512 * t,
                        [p0.ap[0], [128, D], [DF, 2], [1, DF]])
                nc.vector.tensor_tensor(po, in0, in1, Op.mult)

            # tree-reduce over jj: per (tile,i) 128-block: h halves, then jl
            a0 = AP(p0.tensor, p0.offset, [p0.ap[0], [128, 32], [1, 64]])
            a1 = AP(p0.tensor, p0.offset + 64, [p0.ap[0], [128, 32], [1, 64]])
            t1v = t1[:]
            o1 = AP(t1v.tensor, t1v.offset, [t1v.ap[0], [64, 32], [1, 64]])
            nc.vector.tensor_tensor(o1, a0, a1, Op.add)

            b0 = AP(t1v.tensor, t1v.offset, [t1v.ap[0], [64, 32], [1, 32]])
            b1 = AP(t1v.tensor, t1v.offset + 32,
                    [t1v.ap[0], [64, 32], [1, 32]])
            t2v = t2[:]
            o2 = AP(t2v.tensor, t2v.offset, [t2v.ap[0], [32, 32], [1, 32]])
            nc.vector.tensor_tensor(o2, b0, b1, Op.add)

            c0 = AP(t2v.tensor, t2v.offset, [t2v.ap[0], [32, 32], [1, 16]])
            c1 = AP(t2v.tensor, t2v.offset + 16,
                    [t2v.ap[0], [32, 32], [1, 16]])
            ddv = dd[:]
            o3 = AP(ddv.tensor, ddv.offset, [ddv.ap[0], [16, 32], [1, 16]])
            nc.vector.tensor_tensor(o3, c0, c1, Op.add)

            nc.scalar.activation(sq[:], dd[:], Act.Square,
                                 accum_out=acc[:, g:g + 1])

        for k in range(NGATH):
            gather(k)
            if k >= 1:
                compute(2 * k - 2)
                compute(2 * k - 1)
        compute(NG - 2)
        compute(NG - 1)

        colsum = pp.tile([128, 1], f32, tag="colsum")
        ones = pp.tile([128, 1], f32, tag="ones")
        nc.vector.reduce_sum(out=colsum[:], in_=acc[:],
                             axis=mybir.AxisListType.X)
        nc.vector.memset(ones[:], 1.0)
        pt = psp.tile([1, 1], f32, tag="pt")
        nc.tensor.matmul(pt[:], lhsT=colsum[:], rhs=ones[:],
                         start=True, stop=True)
        lsb = pp.tile([1, 1], f32, tag="lsb")
        # *2: each undirected pair contributes both directed edges equally
        nc.vector.tensor_scalar(lsb[:], pt[:], 2.0, None, Op.mult)
        nc.sync.dma_start(loss_d[:], lsb[:])

    nc.compile()
    return nc


_CACHED = {}


def _get_program():
    if "nc" not in _CACHED:
        _inject_axon_hooks()
        _CACHED["nc"] = _build_program()
    return _CACHED["nc"]


def _bf16(a):
    import ml_dtypes
    return a.astype(ml_dtypes.bfloat16)


def _prep_core_inputs(maps3d, src, dst, core):
    """Per-core layout transforms. Returns dict or None if class overflow."""
    e0 = core * EPC
    e1 = e0 + EPC
    d = dst[e0:e1]
    s = src[e0:e1]
    A = maps3d[H + e0:H + e1]
    B = maps3d[e0:e1]

    cls = (d & 1) * 2 + (s & 1)
    eidx = np.full(NT * 128, -1, np.int64)
    for q in range(4):
        iq = np.flatnonzero(cls == q)
        if len(iq) > CB_EDGES:
            return None
        eidx[q * CB_EDGES:q * CB_EDGES + len(iq)] = iq
    valid = eidx >= 0
    ev = eidx[valid]

    m8 = np.zeros((NT * 128, D, 8), np.float32)
    m8[valid, :, :4] = A[ev]
    m8[valid, :, 4:] = -B[ev]
    mcat = _bf16(m8.reshape(NT, 128, 32).transpose(1, 0, 2)
                 .reshape(128, NT * 32))

    dstP = np.zeros(NT * 128, np.int64)
    dstP[valid] = d[ev]
    srcP = np.zeros(NT * 128, np.int64)
    srcP[valid] = s[ev]
    lin = np.empty((NT, 2, 128), np.int16)
    lin[:, 0, :] = (dstP >> 1).reshape(NT, 128)
    lin[:, 1, :] = (srcP >> 1).reshape(NT, 128)
    gidx = np.tile(lin.reshape(-1, 16).T, (8, 1))

    return {
        "mcat": np.ascontiguousarray(mcat),
        "gidx": np.ascontiguousarray(gidx),
    }


def _make_in_maps(x, restriction_maps, edge_index):
    """Build per-core input maps (shared xpair included). None on overflow."""
    x_flat = x.reshape(N, DF).astype(np.float32)
    xp = np.zeros((NPAIR, 2 * DF), np.float32)
    xp[:N // 2, :DF] = x_flat[0::2]
    xp[:N // 2, DF:] = x_flat[1::2]
    xpair = _bf16(xp)
    maps3d = restriction_maps.astype(np.float32)
    src = edge_index[0].astype(np.int64)
    dst = edge_index[1].astype(np.int64)
    in_maps = []
    for c in range(NCORES):
        m = _prep_core_inputs(maps3d, src, dst, c)
        if m is None:
            return None
        m["xpair"] = xpair
        in_maps.append(m)
    return in_maps


def _symmetric_structure(rev_idx):
    r = np.asarray(rev_idx)
    if r.shape != (E,):
        return False
    h = np.arange(H, dtype=r.dtype)
    return bool(np.array_equal(r[:H], h + H) and np.array_equal(r[H:], h))


def _fallback_numpy(x, restriction_maps, edge_index, rev_idx):
    x = np.asarray(x, np.float32)
    maps = np.asarray(restriction_maps, np.float32)
    ei = np.asarray(edge_index)
    rv = np.asarray(rev_idx)
    total = np.float64(0.0)
    chunk = 131072
    ne = ei.shape[1]
    for st in range(0, ne, chunk):
        e = min(st + chunk, ne)
        srcc = ei[0, st:e]
        tgt = ei[1, st:e]
        fvu = maps[rv[st:e]]
        fuv = maps[st:e]
        t1 = np.einsum("eij,ejf->eif", fvu, x[tgt])
        t2 = np.einsum("eij,ejf->eif", fuv, x[srcc])
        dd = t1 - t2
        total += np.sum((dd * dd).astype(np.float64))
    return np.float32(total)


def kernel(x, restriction_maps, edge_index, rev_idx):
    x = np.asarray(x)
    restriction_maps = np.asarray(restriction_maps)
    edge_index = np.asarray(edge_index)
    rev_idx = np.asarray(rev_idx)

    if (x.shape != (N, D, F) or restriction_maps.shape != (E, D, D)
            or edge_index.shape != (2, E) or not _symmetric_structure(rev_idx)):
        return _fallback_numpy(x, restriction_maps, edge_index, rev_idx)

    in_maps = _make_in_maps(x, restriction_maps, edge_index)
    if in_maps is None:
        return _fallback_numpy(x, restriction_maps, edge_index, rev_idx)

    from concourse.bass_utils import run_bass_kernel_spmd

    nc = _get_program()
    res = run_bass_kernel_spmd(nc, in_maps, core_ids=list(range(NCORES)))
    total = np.float32(0.0)
    for c in range(NCORES):
        total += res.results[c]["loss"][0, 0]
    return np.float32(total)
